# revision 6
# baseline (speedup 1.0000x reference)
"""CTRGC Trainium2 kernel (v4).

Reference computation (per sample n):
  g     = Wg @ x[n] + bg                      [64, T=128, V=25]
  xm    = mean_t x[n]                         [64, 25]
  theta = Wth @ xm + bth ;  phi = Wph @ xm + bph        [16, 25]
  rel[i,a,b]  = tanh(theta[i,a] - phi[i,b])   [16, 25, 25]
  rel2        = Wr @ rel + br                 [64, 25, 25]
  A_dyn[c,a,b] = (A+PA)[a,b] + alpha*rel2[c,a,b]
  out[c,t,u]  = sum_v g[c,t,v] * A_dyn[c,u,v]

Sharding: data-parallel over N=128 samples across 8 cores (16 each),
processed on-core in pairs (2x64 channels = 128 partitions).

v4: 3-stage software pipeline with readiness-ordered engine FIFOs.
Iteration i emits (pair indices differ per stage):
  g/XBAR/xsum for pair i+1   (inputs prefetched 2 pairs ahead)
  rel/tanh for pair i        (ready at iteration start)
  step7 block-diag matmuls for pair i-1
  theta/phi for pair i+1
  rel2/advu/transposes/scatter for pair i
step7 is 32 matmuls/pair (4 channels each): stationary = gt4 block
[128,128], moving = BD block-diag view [128,100], built by 4
partition-shifting scatter DMAs into persistent pre-zeroed buffers.
The bias slot is derived from an early reduce of rel (relrowsum)
fused into the rel2 matmul, shortening the per-pair critical chain.
Sync queue carries only the XBAR; scatter on gpsimd; stores on
scalar. Constant-region memsets run once per physical buffer.
"""

import os
import sys

import numpy as np

sys.path.insert(0, "/opt/trn_rl_repo")

import concourse.bass as bass  # noqa: E402
import concourse.tile as tile  # noqa: E402
from concourse import bacc  # noqa: E402
from concourse import mybir  # noqa: E402
from concourse.bass_utils import run_bass_kernel_spmd  # noqa: E402

F32 = mybir.dt.float32
BF16 = mybir.dt.bfloat16

N, C_IN, C_OUT, C_INT, T, V = 128, 64, 64, 16, 128, 25
NCORES = 8
NSH = N // NCORES          # samples per core (16)
NPAIR = NSH // 2           # pairs per core (8)
TV = T * V                 # 3200
CU = C_OUT * V             # 1600
V1 = V + 1                 # 26: v plus the bias slot

_cache = {}


def _build_nc():
    nc = bacc.Bacc("TRN2", target_bir_lowering=False, debug=False)

    # x pre-permuted on host to v-major: [NSH, C_IN, (v,t)]
    xs_d = nc.dram_tensor("xs", [NSH, C_IN, TV], F32, kind="ExternalInput")
    # out stored [T, (b,cb,u)] bf16 per sample; host fixes layout
    ys_d = nc.dram_tensor("ys", [NSH, T, CU], BF16, kind="ExternalOutput")

    ca_d = nc.dram_tensor("constsA", [128, 680], F32, kind="ExternalInput")
    cb_d = nc.dram_tensor("constsB", [128, 448], BF16, kind="ExternalInput")

    with tile.TileContext(nc) as tc:
        _body(nc, tc, xs_d, ys_d, ca_d, cb_d)
    nc.finalize()
    return nc


def _body(nc, tc, xs_d, ys_d, ca_d, cb_d):
    from contextlib import ExitStack
    ctx = ExitStack()
    with ctx:
        const = ctx.enter_context(tc.tile_pool(name="const", bufs=1))
        xin = ctx.enter_context(tc.tile_pool(name="xin", bufs=3))
        gttp = ctx.enter_context(tc.tile_pool(name="gtt", bufs=2))
        gt4p = ctx.enter_context(tc.tile_pool(name="gt4", bufs=3))
        adp = ctx.enter_context(tc.tile_pool(name="ad", bufs=2))
        outp = ctx.enter_context(tc.tile_pool(name="outs", bufs=2))
        smallp = ctx.enter_context(tc.tile_pool(name="small", bufs=3))

        psg = ctx.enter_context(tc.tile_pool(name="psg", bufs=2, space="PSUM"))
        ps7 = ctx.enter_context(tc.tile_pool(name="ps7", bufs=2, space="PSUM"))
        psaux = ctx.enter_context(tc.tile_pool(name="psaux", bufs=3,
                                               space="PSUM"))
        psadt = ctx.enter_context(tc.tile_pool(name="psadt", bufs=1,
                                               space="PSUM"))

        cA = const.tile([128, 680], F32)
        nc.sync.dma_start(cA[:], ca_d[:])
        cB = const.tile([128, 448], BF16)
        nc.sync.dma_start(cB[:], cb_d[:])

        bgp = cA[:, 0:1]               # permuted
        bthp = cA[0:32, 1:2]
        bphp = cA[0:32, 2:3]
        strepA = cA[:, 4:654]          # permuted rows: S[u,v] + a*br[c]
        constS2 = cA[:, 654:679]       # rowsumS[u] + 25*a*br[c] (permuted)
        wgT = cB[:, 0:128]
        wthT = cB[:, 128:160]
        wphT = cB[:, 160:192]
        wrTa = cB[0:32, 192:320]       # permuted cols
        tident = cB[:, 320:448]

        # two persistent block-diag operand buffers; off-block zeros are
        # written once here and never dirtied (scatter writes only the
        # in-block regions each pair)
        bd0 = const.tile([128, 3200], BF16, name="bd0")
        bd1 = const.tile([128, 3200], BF16, name="bd1")
        nc.gpsimd.memset(bd0[:], 0.0)
        nc.gpsimd.memset(bd1[:], 0.0)
        bds = [bd0, bd1]

        cictr = [0]

        def _copy(out_ap, in_ap):
            # alternate DVE/ACT so neither copy engine becomes the pole
            i = cictr[0]
            cictr[0] += 1
            if i % 2 == 1:
                nc.scalar.copy(out_ap, in_ap)
            else:
                nc.vector.tensor_copy(out_ap, in_ap)

        def load_x(p):
            xp = xin.tile([128, TV], BF16, tag="xp")
            nc.gpsimd.dma_start(
                xp[:], xs_d[2 * p:2 * p + 2].rearrange("n c f -> (n c) f"))
            return xp

        st = {}  # cross-stage tile refs keyed by pair index

        def stage_g(p):
            # prefetch x for pair p+1 (consumed next iteration)
            if p + 1 < NPAIR:
                st[("xp", p + 1)] = load_x(p + 1)
            xp = st.pop(("xp", p))

            # ---- g^T direct: per v, out[t, (s,c)] ; gtt [t, (c',v32)] ----
            gtt = gttp.tile([128, 128 * 32], BF16, tag="gtt")
            gtt_v = gtt[:].rearrange("p (c v) -> p v c", v=32)
            if p < 2:
                # constant slots, written once per physical buffer:
                # v=25 <- 1.0 (ones row for the k=26 bias term); v>=26 <- 0
                nc.gpsimd.memset(gtt_v[:, V:V + 1, :], 1.0)
                nc.gpsimd.memset(gtt_v[:, V + 1:32, :], 0.0)
            for v0, vn in _chunks(V, 4):
                gps = psg.tile([128, 512], F32, tag="gps")
                gps_v = gps[:, 0:vn * 128].rearrange(
                    "p (c v) -> p v c", v=vn)
                for vi in range(vn):
                    nc.tensor.matmul(
                        gps_v[:, vi, :],
                        xp[:, (v0 + vi) * T:(v0 + vi + 1) * T], wgT,
                        start=True, stop=True)
                _copy(gtt[:].rearrange(
                          "p (c v) -> p c v", v=32)[:, :, v0:v0 + vn],
                      gps[:, 0:vn * 128].rearrange(
                          "p (c v) -> p c v", v=vn))

            # ---- XBAR: gtt [t, (c',v32)] -> gt4 [(q,v32), (b, t)] ----
            gt4 = gt4p.tile([128, 32 * 128], BF16, tag="gt4")
            nc.sync.dma_start_transpose(
                out=gt4[:].rearrange("p (g t) -> p g t", t=128),
                in_=gtt[:])
            st[("gt4", p)] = gt4

            # ---- xsum over t (v-major: unit stride) ----
            xsum = smallp.tile([128, V], F32, tag="xsum")
            nc.vector.tensor_reduce(
                out=xsum[:], in_=xp[:].rearrange("p (v t) -> p v t", v=V),
                axis=mybir.AxisListType.X, op=mybir.AluOpType.add)
            st[("xsum", p)] = xsum

        def stage_rel(p):
            # ready at iteration start: th/ph were produced last iteration
            th = st.pop(("th", p))
            ph = st.pop(("ph", p))
            reld = smallp.tile([32, V * V1], F32, tag="reld")
            r3 = reld[:].rearrange("p (u v) -> p u v", v=V1)
            th_b = th[:].rearrange("p (u o) -> p u o", o=1).broadcast_to(
                [32, V, V])
            ph_b = ph[:].rearrange("p (o v) -> p o v", o=1).broadcast_to(
                [32, V, V])
            nc.gpsimd.tensor_tensor(
                out=r3[:, :, 0:V], in0=th_b, in1=ph_b,
                op=mybir.AluOpType.subtract)
            # rel + relrowsum packed for the fused rel2 matmul
            relt = smallp.tile([32, V * V1 + V], BF16, tag="relt")
            nc.scalar.activation(
                relt[:, 0:V * V1], reld[:],
                mybir.ActivationFunctionType.Tanh)
            relrs = smallp.tile([32, V], F32, tag="relrs")
            nc.vector.tensor_reduce(
                out=relrs[:], in_=relt[:, 0:V * V1].rearrange(
                    "p (u v) -> p u v", v=V1)[:, :, 0:V],
                axis=mybir.AxisListType.X, op=mybir.AluOpType.add)
            nc.gpsimd.tensor_copy(relt[:, V * V1:V * V1 + V], relrs[:])
            st[("relt", p)] = relt

        def stage_thph(p):
            xsum = st.pop(("xsum", p))
            xsumb = smallp.tile([128, V], BF16, tag="xsumb")
            nc.gpsimd.tensor_copy(xsumb[:], xsum[:])
            thps = psaux.tile([128, 512], F32, tag="auxps")
            nc.tensor.matmul(thps[0:32, 0:V], wthT, xsumb[:],
                             start=True, stop=True)
            th = smallp.tile([32, V], F32, tag="th")
            nc.scalar.activation(th[:], thps[0:32, 0:V],
                                 mybir.ActivationFunctionType.Identity,
                                 bias=bthp)
            phps = psaux.tile([128, 512], F32, tag="auxps")
            nc.tensor.matmul(phps[0:32, 0:V], wphT, xsumb[:],
                             start=True, stop=True)
            ph = smallp.tile([32, V], F32, tag="ph")
            nc.scalar.activation(ph[:], phps[0:32, 0:V],
                                 mybir.ActivationFunctionType.Identity,
                                 bias=bphp)
            st[("th", p)] = th
            st[("ph", p)] = ph

        def stage_adv(p):
            relt = st.pop(("relt", p))
            # ---- rel2 -> advu [p, (u, v26)] = A_dyn[c,u,v] (bf16) ----
            # partition p = (c%4)*32 + c//4 (wrTa cols, strepA rows
            # host-permuted); cols 650..675 carry alpha*Wr@relrowsum
            advu = adp.tile([128, V * V1], BF16, tag="advu")
            r2l = []
            for c0, cn in _chunks(V * V1 + V, 512):
                r2ps = psaux.tile([128, 512], F32, tag="auxps")
                nc.tensor.matmul(r2ps[:, 0:cn], wrTa, relt[:, c0:c0 + cn],
                                 start=True, stop=True)
                r2l.append((c0, cn, r2ps))
            for c0, cn, r2ps in r2l:
                an = min(cn, V * V1 - c0)
                nc.vector.tensor_tensor(
                    out=advu[:, c0:c0 + an], in0=r2ps[:, 0:an],
                    in1=strepA[:, c0:c0 + an], op=mybir.AluOpType.add)
            # bias slot v=25: bg*(rowsumS + 25*a*br + a*Wr@relrowsum)
            c0, cn, r2ps = r2l[-1]
            asum = r2ps[:, V * V1 - c0:V * V1 - c0 + V]
            t1 = smallp.tile([128, V], F32, tag="t1")
            nc.vector.tensor_tensor(out=t1[:], in0=asum, in1=constS2,
                                    op=mybir.AluOpType.add)
            ad3 = advu[:].rearrange("p (u v) -> p u v", v=V1)
            nc.gpsimd.tensor_scalar(
                out=ad3[:, :, V:V1],
                in0=t1[:].rearrange("p (u o) -> p u o", o=1),
                scalar1=bgp, scalar2=None, op0=mybir.AluOpType.mult)

            # ---- adtt [v26, (u, p)] via PE transposes ----
            adtt = adp.tile([128, 128 * V], BF16, tag="adtt")
            for u0, un in _chunks(V, 8):
                atps = psadt.tile([32, 1024], BF16, tag="atps")
                for ui in range(un):
                    nc.tensor.transpose(
                        atps[0:V1, ui * 128:ui * 128 + 128],
                        advu[:, (u0 + ui) * V1:(u0 + ui + 1) * V1], tident)
                _copy(adtt[0:V1, u0 * 128:(u0 + un) * 128],
                      atps[0:V1, 0:un * 128])

            # ---- scatter: adtt -> BD block-diag (partition shift) ----
            bd = bds[p % 2]
            a3 = adtt[0:V1, :].rearrange("p (u c) -> p u c", c=128)
            for cb in range(4):
                src = a3[:, :, cb * 32:(cb + 1) * 32]
                dst = bd[cb * 32:cb * 32 + V1,
                         cb * 800:cb * 800 + 800].rearrange(
                             "p (u b) -> p u b", b=32)
                nc.gpsimd.dma_start(dst, src)

        def stage_out(p):
            # ---- step7: 32 block-diag matmuls, 4 channels each ----
            gt4 = st.pop(("gt4", p))
            bd = bds[p % 2]
            bd_v = bd[:].rearrange("p (c u b) -> p b c u", c=4, u=V, b=32)
            out_sb = outp.tile([128, 2 * CU], BF16, tag="outsb")
            for grp in range(8):
                p7 = ps7.tile([128, 400], F32, tag="p7")
                for j in range(4):
                    b = 4 * grp + j
                    nc.tensor.matmul(
                        p7[:, j * 100:(j + 1) * 100],
                        gt4[:, b * 128:(b + 1) * 128],
                        bd_v[:, b, :, :],
                        start=True, stop=True)
                _copy(out_sb[:, grp * 400:(grp + 1) * 400], p7[:, 0:400])
            # ---- store: per sample, contiguous [t, (b,cb,u)] rows ----
            # (sync queue carries only the XBAR, so stores never block
            # the scalar-engine ACT chain)
            for s in range(2):
                nc.sync.dma_start(ys_d[2 * p + s],
                                  out_sb[:, s * CU:(s + 1) * CU])

        st[("xp", 0)] = load_x(0)
        for i in range(NPAIR + 1):
            if i < NPAIR:
                stage_g(i)
            if i - 1 >= 0:
                stage_rel(i - 1)
            if i - 2 >= 0:
                stage_out(i - 2)
            if i < NPAIR:
                stage_thph(i)
            if i - 1 >= 0:
                stage_adv(i - 1)
        stage_out(NPAIR - 1)


def _chunks(total, step):
    out = []
    s = 0
    while s < total:
        out.append((s, min(step, total - s)))
        s += step
    return out


def _host_params(A, PA, alpha, Wg, bg, Wth, bth, Wph, bph, Wr, br):
    f = np.float32
    al = np.float32(alpha[0])
    # channel permutation: advu partition p holds channel inv(p)
    inv = (np.arange(128) % 32) * 4 + np.arange(128) // 32
    wgT = np.zeros((128, 128), f)
    wgT[:64, :64] = Wg.T
    wgT[64:, 64:] = Wg.T
    # x_mean: fold 1/T into Wth/Wph lhsT
    wthT = np.zeros((128, 32), f)
    wthT[:64, :16] = Wth.T / T
    wthT[64:, 16:] = Wth.T / T
    wphT = np.zeros((128, 32), f)
    wphT[:64, :16] = Wph.T / T
    wphT[64:, 16:] = Wph.T / T
    wrTa = np.zeros((32, 128), f)
    wrTa[:16, :64] = al * Wr.T
    wrTa[16:, 64:] = al * Wr.T
    wrTa = wrTa[:, inv]                     # permute output channels
    bgg = np.concatenate([bg, bg]).astype(f)
    bgp = bgg[inv % 64].reshape(128, 1)
    bthp = np.concatenate([bth, bth]).astype(f).reshape(32, 1)
    bphp = np.concatenate([bph, bph]).astype(f).reshape(32, 1)
    abr = (al * np.concatenate([br, br])).astype(f)
    abrp = abr[inv % 64].reshape(128, 1)
    S = (A + PA).astype(f)
    # strepA[p, u*26+v] = S[u,v] + alpha*br[inv(p)] for v<25; v=25 -> 0
    sU = np.zeros((V, V1), f)
    sU[:, :V] = S
    strepA = np.tile(sU.reshape(1, -1), (128, 1)).astype(f)
    mask = (np.arange(V * V1) % V1 < V).astype(f).reshape(1, -1)
    strepA = strepA + abrp @ mask
    # constS2[p, u] = rowsumS[u] + 25*alpha*br[inv(p)]
    constS2 = (np.tile(S.sum(axis=1).reshape(1, V), (128, 1))
               + V * abrp).astype(f)
    cA = np.zeros((128, 680), f)
    cA[:, 0:1] = bgp
    cA[0:32, 1:2] = bthp
    cA[0:32, 2:3] = bphp
    cA[:, 4:654] = strepA
    cA[:, 654:679] = constS2
    bf16 = __import__("ml_dtypes").bfloat16
    cB = np.zeros((128, 448), f)
    cB[:, 0:128] = wgT
    cB[:, 128:160] = wthT
    cB[:, 160:192] = wphT
    cB[0:32, 192:320] = wrTa
    cB[:, 320:448] = np.eye(128, dtype=f)
    return dict(constsA=cA, constsB=cB.astype(bf16))


def kernel(**inputs):
    x = np.asarray(inputs["x"], np.float32)
    params = _host_params(
        np.asarray(inputs["A"], np.float32), np.asarray(inputs["PA"], np.float32),
        np.asarray(inputs["alpha"], np.float32), np.asarray(inputs["Wg"], np.float32),
        np.asarray(inputs["bg"], np.float32), np.asarray(inputs["Wth"], np.float32),
        np.asarray(inputs["bth"], np.float32), np.asarray(inputs["Wph"], np.float32),
        np.asarray(inputs["bph"], np.float32), np.asarray(inputs["Wr"], np.float32),
        np.asarray(inputs["br"], np.float32))

    if "nc" not in _cache:
        _cache["nc"] = _build_nc()
    nc = _cache["nc"]

    # upload x v-major: [NSH, C_IN, (v,t)]
    xv = np.ascontiguousarray(x.transpose(0, 1, 3, 2)).reshape(N, C_IN, TV)
    in_maps = []
    for i in range(NCORES):
        m = {"xs": xv[i * NSH:(i + 1) * NSH]}
        m.update(params)
        in_maps.append(m)

    res = run_bass_kernel_spmd(nc, in_maps, list(range(NCORES)),
                               **_cache.get("run_kwargs", {}))
    # device emits [NSH, T, (b16,cb4,u25)] bf16 per sample (c = 4b+cb)
    out = np.concatenate([np.asarray(res.results[i]["ys"]) for i in range(NCORES)],
                         axis=0)
    out = out.reshape(N, T, 16, 4, V).transpose(0, 2, 3, 1, 4).reshape(
        N, C_OUT, T, V)
    _cache["last_results"] = res
    return np.ascontiguousarray(out, dtype=np.float32)


if __name__ == "__main__":
    nc = _build_nc()
    print("build ok")


# revision 10
# speedup vs baseline: 1.1482x; 1.1482x over previous
"""CTRGC Trainium2 kernel (v4).

Reference computation (per sample n):
  g     = Wg @ x[n] + bg                      [64, T=128, V=25]
  xm    = mean_t x[n]                         [64, 25]
  theta = Wth @ xm + bth ;  phi = Wph @ xm + bph        [16, 25]
  rel[i,a,b]  = tanh(theta[i,a] - phi[i,b])   [16, 25, 25]
  rel2        = Wr @ rel + br                 [64, 25, 25]
  A_dyn[c,a,b] = (A+PA)[a,b] + alpha*rel2[c,a,b]
  out[c,t,u]  = sum_v g[c,t,v] * A_dyn[c,u,v]

Sharding: data-parallel over N=128 samples across 8 cores (16 each),
processed on-core in pairs (2x64 channels = 128 partitions).

v5: 6-stage software pipeline with readiness-ordered engine FIFOs —
every stage's inputs come from a previous iteration, so each engine
FIFO drains without intra-iteration dependency waits. Iteration i
emits: g/XBAR/xsum(i), step7(i-3), theta/phi(i-1), rel/tanh(i-2),
rel2/advu/transposes/scatter(i-2).
step7 is 32 matmuls/pair (4 channels each): stationary = gt4 block
[128,128], moving = BD block-diag view [128,100], built by 4
partition-shifting scatter DMAs into persistent pre-zeroed buffers.
The bias slot is derived from an early reduce of rel (relrowsum)
fused into the rel2 matmul, shortening the per-pair critical chain.
x arrives pre-converted to bf16 (host-side; the g matmul consumed
bf16 anyway) so loads ride the HWDGE queues. Sync carries XBAR +
stores + scatter; constant-region memsets run once per buffer.
"""

import os
import sys

import numpy as np

sys.path.insert(0, "/opt/trn_rl_repo")

import concourse.bass as bass  # noqa: E402
import concourse.tile as tile  # noqa: E402
from concourse import bacc  # noqa: E402
from concourse import mybir  # noqa: E402
from concourse.bass_utils import run_bass_kernel_spmd  # noqa: E402

F32 = mybir.dt.float32
BF16 = mybir.dt.bfloat16

N, C_IN, C_OUT, C_INT, T, V = 128, 64, 64, 16, 128, 25
NCORES = 8
NSH = N // NCORES          # samples per core (16)
NPAIR = NSH // 2           # pairs per core (8)
TV = T * V                 # 3200
CU = C_OUT * V             # 1600
V1 = V + 1                 # 26: v plus the bias slot

_cache = {}


def _build_nc():
    nc = bacc.Bacc("TRN2", target_bir_lowering=False, debug=False)

    # x pre-permuted on host to v-major: [NSH, C_IN, (v,t)]
    xs_d = nc.dram_tensor("xs", [NSH, C_IN, TV], BF16, kind="ExternalInput")
    # out stored [T, (b,cb,u)] bf16 per sample; host fixes layout
    ys_d = nc.dram_tensor("ys", [NSH, T, CU], BF16, kind="ExternalOutput")

    ca_d = nc.dram_tensor("constsA", [128, 680], F32, kind="ExternalInput")
    cb_d = nc.dram_tensor("constsB", [128, 448], BF16, kind="ExternalInput")

    with tile.TileContext(nc) as tc:
        _body(nc, tc, xs_d, ys_d, ca_d, cb_d)
    nc.finalize()
    return nc


def _body(nc, tc, xs_d, ys_d, ca_d, cb_d):
    from contextlib import ExitStack
    ctx = ExitStack()
    with ctx:
        const = ctx.enter_context(tc.tile_pool(name="const", bufs=1))
        xin = ctx.enter_context(tc.tile_pool(name="xin", bufs=3))
        gttp = ctx.enter_context(tc.tile_pool(name="gtt", bufs=2))
        gt4p = ctx.enter_context(tc.tile_pool(name="gt4", bufs=4))
        adp = ctx.enter_context(tc.tile_pool(name="ad", bufs=2))
        outp = ctx.enter_context(tc.tile_pool(name="outs", bufs=2))
        smallp = ctx.enter_context(tc.tile_pool(name="small", bufs=3))

        psg = ctx.enter_context(tc.tile_pool(name="psg", bufs=2, space="PSUM"))
        ps7 = ctx.enter_context(tc.tile_pool(name="ps7", bufs=2, space="PSUM"))
        psaux = ctx.enter_context(tc.tile_pool(name="psaux", bufs=3,
                                               space="PSUM"))
        psadt = ctx.enter_context(tc.tile_pool(name="psadt", bufs=1,
                                               space="PSUM"))

        cA = const.tile([128, 680], F32)
        nc.sync.dma_start(cA[:], ca_d[:])
        cB = const.tile([128, 448], BF16)
        nc.sync.dma_start(cB[:], cb_d[:])

        bgp = cA[:, 0:1]               # permuted
        bthp = cA[0:32, 1:2]
        bphp = cA[0:32, 2:3]
        strepA = cA[:, 4:654]          # permuted rows: S[u,v] + a*br[c]
        constS2 = cA[:, 654:679]       # rowsumS[u] + 25*a*br[c] (permuted)
        wgT = cB[:, 0:128]
        wthT = cB[:, 128:160]
        wphT = cB[:, 160:192]
        wrTa = cB[0:32, 192:320]       # permuted cols
        tident = cB[:, 320:448]

        # two persistent block-diag operand buffers; off-block zeros are
        # written once here and never dirtied (scatter writes only the
        # in-block regions each pair)
        bd0 = const.tile([128, 3200], BF16, name="bd0")
        bd1 = const.tile([128, 3200], BF16, name="bd1")
        nc.gpsimd.memset(bd0[:], 0.0)
        nc.gpsimd.memset(bd1[:], 0.0)
        bds = [bd0, bd1]

        cictr = [0]

        def _copy(out_ap, in_ap):
            # alternate DVE/ACT so neither copy engine becomes the pole
            i = cictr[0]
            cictr[0] += 1
            if i % 2 == 1:
                nc.scalar.copy(out_ap, in_ap)
            else:
                nc.vector.tensor_copy(out_ap, in_ap)

        def load_x(p):
            xp = xin.tile([128, TV], BF16, tag="xp")
            nc.scalar.dma_start(
                xp[:], xs_d[2 * p:2 * p + 2].rearrange("n c f -> (n c) f"))
            return xp

        st = {}  # cross-stage tile refs keyed by pair index

        def stage_g(p):
            # prefetch x two pairs ahead
            if p + 2 < NPAIR:
                st[("xp", p + 2)] = load_x(p + 2)
            xp = st.pop(("xp", p))

            # ---- g^T direct: per v, out[t, (s,c)] ; gtt [t, (c',v32)] ----
            gtt = gttp.tile([128, 128 * 32], BF16, tag="gtt")
            gtt_v = gtt[:].rearrange("p (c v) -> p v c", v=32)
            if p < 2:
                # constant slots, written once per physical buffer:
                # v=25 <- 1.0 (ones row for the k=26 bias term); v>=26 <- 0
                nc.gpsimd.memset(gtt_v[:, V:V + 1, :], 1.0)
                nc.gpsimd.memset(gtt_v[:, V + 1:32, :], 0.0)
            for v0, vn in _chunks(V, 4):
                gps = psg.tile([128, 512], F32, tag="gps")
                gps_v = gps[:, 0:vn * 128].rearrange(
                    "p (c v) -> p v c", v=vn)
                for vi in range(vn):
                    nc.tensor.matmul(
                        gps_v[:, vi, :],
                        xp[:, (v0 + vi) * T:(v0 + vi + 1) * T], wgT,
                        start=True, stop=True)
                _copy(gtt[:].rearrange(
                          "p (c v) -> p c v", v=32)[:, :, v0:v0 + vn],
                      gps[:, 0:vn * 128].rearrange(
                          "p (c v) -> p c v", v=vn))

            # ---- XBAR: gtt [t, (c',v32)] -> gt4 [(q,v32), (b, t)] ----
            gt4 = gt4p.tile([128, 32 * 128], BF16, tag="gt4")
            nc.sync.dma_start_transpose(
                out=gt4[:].rearrange("p (g t) -> p g t", t=128),
                in_=gtt[:])
            st[("gt4", p)] = gt4

            # ---- xsum over t (v-major: unit stride) ----
            xsum = smallp.tile([128, V], F32, tag="xsum")
            nc.vector.tensor_reduce(
                out=xsum[:], in_=xp[:].rearrange("p (v t) -> p v t", v=V),
                axis=mybir.AxisListType.X, op=mybir.AluOpType.add)
            st[("xsum", p)] = xsum

        def stage_rel(p):
            # ready at iteration start: th/ph were produced last iteration
            th = st.pop(("th", p))
            ph = st.pop(("ph", p))
            reld = smallp.tile([32, V * V1], F32, tag="reld")
            r3 = reld[:].rearrange("p (u v) -> p u v", v=V1)
            th_b = th[:].rearrange("p (u o) -> p u o", o=1).broadcast_to(
                [32, V, V])
            ph_b = ph[:].rearrange("p (o v) -> p o v", o=1).broadcast_to(
                [32, V, V])
            nc.gpsimd.tensor_tensor(
                out=r3[:, :, 0:V], in0=th_b, in1=ph_b,
                op=mybir.AluOpType.subtract)
            # rel + relrowsum packed for the fused rel2 matmul
            relt = smallp.tile([32, V * V1 + V], BF16, tag="relt")
            nc.scalar.activation(
                relt[:, 0:V * V1], reld[:],
                mybir.ActivationFunctionType.Tanh)
            relrs = smallp.tile([32, V], F32, tag="relrs")
            nc.vector.tensor_reduce(
                out=relrs[:], in_=relt[:, 0:V * V1].rearrange(
                    "p (u v) -> p u v", v=V1)[:, :, 0:V],
                axis=mybir.AxisListType.X, op=mybir.AluOpType.add)
            nc.gpsimd.tensor_copy(relt[:, V * V1:V * V1 + V], relrs[:])
            st[("relt", p)] = relt

        def stage_thph(p):
            xsum = st.pop(("xsum", p))
            xsumb = smallp.tile([128, V], BF16, tag="xsumb")
            nc.gpsimd.tensor_copy(xsumb[:], xsum[:])
            thps = psaux.tile([128, 512], F32, tag="auxps")
            nc.tensor.matmul(thps[0:32, 0:V], wthT, xsumb[:],
                             start=True, stop=True)
            th = smallp.tile([32, V], F32, tag="th")
            nc.scalar.activation(th[:], thps[0:32, 0:V],
                                 mybir.ActivationFunctionType.Identity,
                                 bias=bthp)
            phps = psaux.tile([128, 512], F32, tag="auxps")
            nc.tensor.matmul(phps[0:32, 0:V], wphT, xsumb[:],
                             start=True, stop=True)
            ph = smallp.tile([32, V], F32, tag="ph")
            nc.scalar.activation(ph[:], phps[0:32, 0:V],
                                 mybir.ActivationFunctionType.Identity,
                                 bias=bphp)
            st[("th", p)] = th
            st[("ph", p)] = ph

        def stage_adv(p):
            relt = st.pop(("relt", p))
            # ---- rel2 -> advu [p, (u, v26)] = A_dyn[c,u,v] (bf16) ----
            # partition p = (c%4)*32 + c//4 (wrTa cols, strepA rows
            # host-permuted); cols 650..675 carry alpha*Wr@relrowsum
            advu = adp.tile([128, V * V1], BF16, tag="advu")
            r2l = []
            for c0, cn in _chunks(V * V1 + V, 512):
                r2ps = psaux.tile([128, 512], F32, tag="auxps")
                nc.tensor.matmul(r2ps[:, 0:cn], wrTa, relt[:, c0:c0 + cn],
                                 start=True, stop=True)
                r2l.append((c0, cn, r2ps))
            for c0, cn, r2ps in r2l:
                an = min(cn, V * V1 - c0)
                nc.vector.tensor_tensor(
                    out=advu[:, c0:c0 + an], in0=r2ps[:, 0:an],
                    in1=strepA[:, c0:c0 + an], op=mybir.AluOpType.add)
            # bias slot v=25: bg*(rowsumS + 25*a*br + a*Wr@relrowsum)
            c0, cn, r2ps = r2l[-1]
            asum = r2ps[:, V * V1 - c0:V * V1 - c0 + V]
            t1 = smallp.tile([128, V], F32, tag="t1")
            nc.vector.tensor_tensor(out=t1[:], in0=asum, in1=constS2,
                                    op=mybir.AluOpType.add)
            ad3 = advu[:].rearrange("p (u v) -> p u v", v=V1)
            nc.gpsimd.tensor_scalar(
                out=ad3[:, :, V:V1],
                in0=t1[:].rearrange("p (u o) -> p u o", o=1),
                scalar1=bgp, scalar2=None, op0=mybir.AluOpType.mult)

            # ---- adtt [v26, (u, p)] via PE transposes ----
            adtt = adp.tile([128, 128 * V], BF16, tag="adtt")
            for u0, un in _chunks(V, 8):
                atps = psadt.tile([32, 1024], BF16, tag="atps")
                for ui in range(un):
                    nc.tensor.transpose(
                        atps[0:V1, ui * 128:ui * 128 + 128],
                        advu[:, (u0 + ui) * V1:(u0 + ui + 1) * V1], tident)
                _copy(adtt[0:V1, u0 * 128:(u0 + un) * 128],
                      atps[0:V1, 0:un * 128])

            # ---- scatter: adtt -> BD block-diag (partition shift) ----
            bd = bds[p % 2]
            a3 = adtt[0:V1, :].rearrange("p (u c) -> p u c", c=128)
            for cb in range(4):
                src = a3[:, :, cb * 32:(cb + 1) * 32]
                dst = bd[cb * 32:cb * 32 + V1,
                         cb * 800:cb * 800 + 800].rearrange(
                             "p (u b) -> p u b", b=32)
                nc.sync.dma_start(dst, src)

        def stage_out(p):
            # ---- step7: 32 block-diag matmuls, 4 channels each ----
            gt4 = st.pop(("gt4", p))
            bd = bds[p % 2]
            bd_v = bd[:].rearrange("p (c u b) -> p b c u", c=4, u=V, b=32)
            out_sb = outp.tile([128, 2 * CU], BF16, tag="outsb")
            for grp in range(8):
                p7 = ps7.tile([128, 400], F32, tag="p7")
                for j in range(4):
                    b = 4 * grp + j
                    nc.tensor.matmul(
                        p7[:, j * 100:(j + 1) * 100],
                        gt4[:, b * 128:(b + 1) * 128],
                        bd_v[:, b, :, :],
                        start=True, stop=True)
                _copy(out_sb[:, grp * 400:(grp + 1) * 400], p7[:, 0:400])
            # ---- store: per sample, contiguous [t, (b,cb,u)] rows ----
            # (sync queue carries only the XBAR, so stores never block
            # the scalar-engine ACT chain)
            for s in range(2):
                nc.sync.dma_start(ys_d[2 * p + s],
                                  out_sb[:, s * CU:(s + 1) * CU])

        st[("xp", 0)] = load_x(0)
        st[("xp", 1)] = load_x(1)
        for i in range(NPAIR + 3):
            if i < NPAIR:
                stage_g(i)
            if 0 <= i - 3 < NPAIR:
                stage_out(i - 3)
            if 0 <= i - 1 < NPAIR:
                stage_thph(i - 1)
            if 0 <= i - 2 < NPAIR:
                stage_rel(i - 2)
            if 0 <= i - 2 < NPAIR:
                stage_adv(i - 2)


def _chunks(total, step):
    out = []
    s = 0
    while s < total:
        out.append((s, min(step, total - s)))
        s += step
    return out


def _host_params(A, PA, alpha, Wg, bg, Wth, bth, Wph, bph, Wr, br):
    f = np.float32
    al = np.float32(alpha[0])
    # channel permutation: advu partition p holds channel inv(p)
    inv = (np.arange(128) % 32) * 4 + np.arange(128) // 32
    wgT = np.zeros((128, 128), f)
    wgT[:64, :64] = Wg.T
    wgT[64:, 64:] = Wg.T
    # x_mean: fold 1/T into Wth/Wph lhsT
    wthT = np.zeros((128, 32), f)
    wthT[:64, :16] = Wth.T / T
    wthT[64:, 16:] = Wth.T / T
    wphT = np.zeros((128, 32), f)
    wphT[:64, :16] = Wph.T / T
    wphT[64:, 16:] = Wph.T / T
    wrTa = np.zeros((32, 128), f)
    wrTa[:16, :64] = al * Wr.T
    wrTa[16:, 64:] = al * Wr.T
    wrTa = wrTa[:, inv]                     # permute output channels
    bgg = np.concatenate([bg, bg]).astype(f)
    bgp = bgg[inv % 64].reshape(128, 1)
    bthp = np.concatenate([bth, bth]).astype(f).reshape(32, 1)
    bphp = np.concatenate([bph, bph]).astype(f).reshape(32, 1)
    abr = (al * np.concatenate([br, br])).astype(f)
    abrp = abr[inv % 64].reshape(128, 1)
    S = (A + PA).astype(f)
    # strepA[p, u*26+v] = S[u,v] + alpha*br[inv(p)] for v<25; v=25 -> 0
    sU = np.zeros((V, V1), f)
    sU[:, :V] = S
    strepA = np.tile(sU.reshape(1, -1), (128, 1)).astype(f)
    mask = (np.arange(V * V1) % V1 < V).astype(f).reshape(1, -1)
    strepA = strepA + abrp @ mask
    # constS2[p, u] = rowsumS[u] + 25*alpha*br[inv(p)]
    constS2 = (np.tile(S.sum(axis=1).reshape(1, V), (128, 1))
               + V * abrp).astype(f)
    cA = np.zeros((128, 680), f)
    cA[:, 0:1] = bgp
    cA[0:32, 1:2] = bthp
    cA[0:32, 2:3] = bphp
    cA[:, 4:654] = strepA
    cA[:, 654:679] = constS2
    bf16 = __import__("ml_dtypes").bfloat16
    cB = np.zeros((128, 448), f)
    cB[:, 0:128] = wgT
    cB[:, 128:160] = wthT
    cB[:, 160:192] = wphT
    cB[0:32, 192:320] = wrTa
    cB[:, 320:448] = np.eye(128, dtype=f)
    return dict(constsA=cA, constsB=cB.astype(bf16))


def kernel(**inputs):
    x = np.asarray(inputs["x"], np.float32)
    params = _host_params(
        np.asarray(inputs["A"], np.float32), np.asarray(inputs["PA"], np.float32),
        np.asarray(inputs["alpha"], np.float32), np.asarray(inputs["Wg"], np.float32),
        np.asarray(inputs["bg"], np.float32), np.asarray(inputs["Wth"], np.float32),
        np.asarray(inputs["bth"], np.float32), np.asarray(inputs["Wph"], np.float32),
        np.asarray(inputs["bph"], np.float32), np.asarray(inputs["Wr"], np.float32),
        np.asarray(inputs["br"], np.float32))

    if "nc" not in _cache:
        _cache["nc"] = _build_nc()
    nc = _cache["nc"]

    # upload x v-major, host-converted to bf16 (the g matmul consumes
    # bf16 either way; this halves x HBM traffic and avoids SWDGE casts)
    bf16 = __import__("ml_dtypes").bfloat16
    xv = np.ascontiguousarray(x.transpose(0, 1, 3, 2)).reshape(
        N, C_IN, TV).astype(bf16)
    in_maps = []
    for i in range(NCORES):
        m = {"xs": xv[i * NSH:(i + 1) * NSH]}
        m.update(params)
        in_maps.append(m)

    res = run_bass_kernel_spmd(nc, in_maps, list(range(NCORES)),
                               **_cache.get("run_kwargs", {}))
    # device emits [NSH, T, (b16,cb4,u25)] bf16 per sample (c = 4b+cb)
    out = np.concatenate([np.asarray(res.results[i]["ys"]) for i in range(NCORES)],
                         axis=0)
    out = out.reshape(N, T, 16, 4, V).transpose(0, 2, 3, 1, 4).reshape(
        N, C_OUT, T, V)
    _cache["last_results"] = res
    return np.ascontiguousarray(out, dtype=np.float32)


if __name__ == "__main__":
    nc = _build_nc()
    print("build ok")


# revision 11
# speedup vs baseline: 1.2237x; 1.0658x over previous
"""CTRGC Trainium2 kernel (v4).

Reference computation (per sample n):
  g     = Wg @ x[n] + bg                      [64, T=128, V=25]
  xm    = mean_t x[n]                         [64, 25]
  theta = Wth @ xm + bth ;  phi = Wph @ xm + bph        [16, 25]
  rel[i,a,b]  = tanh(theta[i,a] - phi[i,b])   [16, 25, 25]
  rel2        = Wr @ rel + br                 [64, 25, 25]
  A_dyn[c,a,b] = (A+PA)[a,b] + alpha*rel2[c,a,b]
  out[c,t,u]  = sum_v g[c,t,v] * A_dyn[c,u,v]

Sharding: data-parallel over N=128 samples across 8 cores (16 each),
processed on-core in pairs (2x64 channels = 128 partitions).

v5: 6-stage software pipeline with readiness-ordered engine FIFOs —
every stage's inputs come from a previous iteration, so each engine
FIFO drains without intra-iteration dependency waits. Iteration i
emits: g/XBAR/xsum(i), step7(i-3), theta/phi(i-1), rel/tanh(i-2),
rel2/advu/transposes/scatter(i-2).
step7 is 32 matmuls/pair (4 channels each): stationary = gt4 block
[128,128], moving = BD block-diag view [128,100], built by 4
partition-shifting scatter DMAs into persistent pre-zeroed buffers.
The bias slot is derived from an early reduce of rel (relrowsum)
fused into the rel2 matmul, shortening the per-pair critical chain.
x arrives pre-converted to bf16 (host-side; the g matmul consumed
bf16 anyway) so loads ride the HWDGE queues. Sync carries XBAR +
stores + scatter; constant-region memsets run once per buffer.
"""

import os
import sys

import numpy as np

sys.path.insert(0, "/opt/trn_rl_repo")

import concourse.bass as bass  # noqa: E402
import concourse.tile as tile  # noqa: E402
from concourse import bacc  # noqa: E402
from concourse import mybir  # noqa: E402
from concourse.bass_utils import run_bass_kernel_spmd  # noqa: E402

F32 = mybir.dt.float32
BF16 = mybir.dt.bfloat16

N, C_IN, C_OUT, C_INT, T, V = 128, 64, 64, 16, 128, 25
NCORES = 8
NSH = N // NCORES          # samples per core (16)
NPAIR = NSH // 2           # pairs per core (8)
TV = T * V                 # 3200
CU = C_OUT * V             # 1600
V1 = V + 1                 # 26: v plus the bias slot

_cache = {}


def _build_nc():
    nc = bacc.Bacc("TRN2", target_bir_lowering=False, debug=False)

    # x pre-permuted on host to v-major: [NSH, C_IN, (v,t)]
    xs_d = nc.dram_tensor("xs", [NSH, C_IN, TV], BF16, kind="ExternalInput")
    # out stored [T, (b,cb,u)] bf16 per sample; host fixes layout
    ys_d = nc.dram_tensor("ys", [NSH, T, CU], BF16, kind="ExternalOutput")

    ca_d = nc.dram_tensor("constsA", [128, 680], F32, kind="ExternalInput")
    cb_d = nc.dram_tensor("constsB", [128, 448], BF16, kind="ExternalInput")

    with tile.TileContext(nc) as tc:
        _body(nc, tc, xs_d, ys_d, ca_d, cb_d)
    nc.finalize()
    return nc


def _body(nc, tc, xs_d, ys_d, ca_d, cb_d):
    from contextlib import ExitStack
    ctx = ExitStack()
    with ctx:
        const = ctx.enter_context(tc.tile_pool(name="const", bufs=1))
        xin = ctx.enter_context(tc.tile_pool(name="xin", bufs=3))
        gttp = ctx.enter_context(tc.tile_pool(name="gtt", bufs=2))
        gt4p = ctx.enter_context(tc.tile_pool(name="gt4", bufs=4))
        adp = ctx.enter_context(tc.tile_pool(name="ad", bufs=2))
        outp = ctx.enter_context(tc.tile_pool(name="outs", bufs=2))
        smallp = ctx.enter_context(tc.tile_pool(name="small", bufs=3))
        xredp = ctx.enter_context(tc.tile_pool(name="xred", bufs=1))

        psg = ctx.enter_context(tc.tile_pool(name="psg", bufs=2, space="PSUM"))
        ps7 = ctx.enter_context(tc.tile_pool(name="ps7", bufs=2, space="PSUM"))
        psaux = ctx.enter_context(tc.tile_pool(name="psaux", bufs=3,
                                               space="PSUM"))
        psadt = ctx.enter_context(tc.tile_pool(name="psadt", bufs=1,
                                               space="PSUM"))

        cA = const.tile([128, 680], F32)
        nc.sync.dma_start(cA[:], ca_d[:])
        cB = const.tile([128, 448], BF16)
        nc.sync.dma_start(cB[:], cb_d[:])

        bgp = cA[:, 0:1]               # permuted
        bthp = cA[0:32, 1:2]
        bphp = cA[0:32, 2:3]
        strepA = cA[:, 4:654]          # permuted rows: S[u,v] + a*br[c]
        constS2 = cA[:, 654:679]       # rowsumS[u] + 25*a*br[c] (permuted)
        wgT = cB[:, 0:128]
        wthT = cB[:, 128:160]
        wphT = cB[:, 160:192]
        wrTa = cB[0:32, 192:320]       # permuted cols
        tident = cB[:, 320:448]

        # two persistent block-diag operand buffers; off-block zeros are
        # written once here and never dirtied (scatter writes only the
        # in-block regions each pair)
        bd0 = const.tile([128, 3200], BF16, name="bd0")
        bd1 = const.tile([128, 3200], BF16, name="bd1")
        nc.gpsimd.memset(bd0[:], 0.0)
        nc.gpsimd.memset(bd1[:], 0.0)
        bds = [bd0, bd1]

        cictr = [0]

        def _copy(out_ap, in_ap):
            # alternate DVE/ACT so neither copy engine becomes the pole
            i = cictr[0]
            cictr[0] += 1
            if i % 2 == 1:
                nc.scalar.copy(out_ap, in_ap)
            else:
                nc.vector.tensor_copy(out_ap, in_ap)

        def load_x(p):
            xp = xin.tile([128, TV], BF16, tag="xp")
            nc.scalar.dma_start(
                xp[:], xs_d[2 * p:2 * p + 2].rearrange("n c f -> (n c) f"))
            return xp

        st = {}  # cross-stage tile refs keyed by pair index

        def stage_g(p):
            # prefetch x two pairs ahead
            if p + 2 < NPAIR:
                st[("xp", p + 2)] = load_x(p + 2)
            xp = st.pop(("xp", p))

            # ---- g^T direct: per v, out[t, (s,c)] ; gtt [t, (c',v32)] ----
            gtt = gttp.tile([128, 128 * 32], BF16, tag="gtt")
            gtt_v = gtt[:].rearrange("p (c v) -> p v c", v=32)
            if p < 2:
                # constant slots, written once per physical buffer:
                # v=25 <- 1.0 (ones row for the k=26 bias term); v>=26 <- 0
                nc.gpsimd.memset(gtt_v[:, V:V + 1, :], 1.0)
                nc.gpsimd.memset(gtt_v[:, V + 1:32, :], 0.0)
            for v0, vn in _chunks(V, 4):
                gps = psg.tile([128, 512], F32, tag="gps")
                gps_v = gps[:, 0:vn * 128].rearrange(
                    "p (c v) -> p v c", v=vn)
                for vi in range(vn):
                    nc.tensor.matmul(
                        gps_v[:, vi, :],
                        xp[:, (v0 + vi) * T:(v0 + vi + 1) * T], wgT,
                        start=True, stop=True)
                _copy(gtt[:].rearrange(
                          "p (c v) -> p c v", v=32)[:, :, v0:v0 + vn],
                      gps[:, 0:vn * 128].rearrange(
                          "p (c v) -> p c v", v=vn))

            # ---- XBAR: gtt [t, (c',v32)] -> gt4 [(q,v32), (b, t)] ----
            gt4 = gt4p.tile([128, 32 * 128], BF16, tag="gt4")
            nc.sync.dma_start_transpose(
                out=gt4[:].rearrange("p (g t) -> p g t", t=128),
                in_=gtt[:])
            st[("gt4", p)] = gt4

            # ---- xsum over t: f32 add-tree on gpsimd (DVE is the
            # busier engine; gpsimd has slack) ----
            xr = xredp.tile([128, 2400], F32, tag="xr")
            xsumb = smallp.tile([128, V], BF16, tag="xsumb")
            half = 64
            src_v = xp[:].rearrange("p (v t) -> p v t", v=V)
            dst_v = xr[:, 0:V * half].rearrange("p (v t) -> p v t", v=V)
            nc.gpsimd.tensor_tensor(
                out=dst_v, in0=src_v[:, :, 0:half], in1=src_v[:, :, half:],
                op=mybir.AluOpType.add)
            off = [0, 1600]
            cur = 0
            while half > 1:
                nh = half // 2
                src_v = xr[:, off[cur]:off[cur] + V * half].rearrange(
                    "p (v t) -> p v t", v=V)
                if nh > 1:
                    dst_v = xr[:, off[1 - cur]:off[1 - cur] + V * nh
                               ].rearrange("p (v t) -> p v t", v=V)
                else:
                    dst_v = xsumb[:].rearrange("p (v t) -> p v t", v=V)
                nc.gpsimd.tensor_tensor(
                    out=dst_v, in0=src_v[:, :, 0:nh],
                    in1=src_v[:, :, nh:], op=mybir.AluOpType.add)
                half = nh
                cur = 1 - cur
            st[("xsumb", p)] = xsumb

        def stage_rel(p):
            # ready at iteration start: th/ph were produced last iteration
            th = st.pop(("th", p))
            ph = st.pop(("ph", p))
            reld = smallp.tile([32, V * V1], F32, tag="reld")
            r3 = reld[:].rearrange("p (u v) -> p u v", v=V1)
            th_b = th[:].rearrange("p (u o) -> p u o", o=1).broadcast_to(
                [32, V, V])
            ph_b = ph[:].rearrange("p (o v) -> p o v", o=1).broadcast_to(
                [32, V, V])
            nc.gpsimd.tensor_tensor(
                out=r3[:, :, 0:V], in0=th_b, in1=ph_b,
                op=mybir.AluOpType.subtract)
            # rel + relrowsum packed for the fused rel2 matmul
            relt = smallp.tile([32, V * V1 + V], BF16, tag="relt")
            nc.scalar.activation(
                relt[:, 0:V * V1], reld[:],
                mybir.ActivationFunctionType.Tanh)
            relrs = smallp.tile([32, V], F32, tag="relrs")
            nc.vector.tensor_reduce(
                out=relrs[:], in_=relt[:, 0:V * V1].rearrange(
                    "p (u v) -> p u v", v=V1)[:, :, 0:V],
                axis=mybir.AxisListType.X, op=mybir.AluOpType.add)
            nc.gpsimd.tensor_copy(relt[:, V * V1:V * V1 + V], relrs[:])
            st[("relt", p)] = relt

        def stage_thph(p):
            xsumb = st.pop(("xsumb", p))
            thps = psaux.tile([128, 512], F32, tag="auxps")
            nc.tensor.matmul(thps[0:32, 0:V], wthT, xsumb[:],
                             start=True, stop=True)
            th = smallp.tile([32, V], F32, tag="th")
            nc.scalar.activation(th[:], thps[0:32, 0:V],
                                 mybir.ActivationFunctionType.Identity,
                                 bias=bthp)
            phps = psaux.tile([128, 512], F32, tag="auxps")
            nc.tensor.matmul(phps[0:32, 0:V], wphT, xsumb[:],
                             start=True, stop=True)
            ph = smallp.tile([32, V], F32, tag="ph")
            nc.scalar.activation(ph[:], phps[0:32, 0:V],
                                 mybir.ActivationFunctionType.Identity,
                                 bias=bphp)
            st[("th", p)] = th
            st[("ph", p)] = ph

        def stage_adv(p):
            relt = st.pop(("relt", p))
            # ---- rel2 -> advu [p, (u, v26)] = A_dyn[c,u,v] (bf16) ----
            # partition p = (c%4)*32 + c//4 (wrTa cols, strepA rows
            # host-permuted); cols 650..675 carry alpha*Wr@relrowsum
            advu = adp.tile([128, V * V1], BF16, tag="advu")
            r2l = []
            for c0, cn in _chunks(V * V1 + V, 512):
                r2ps = psaux.tile([128, 512], F32, tag="auxps")
                nc.tensor.matmul(r2ps[:, 0:cn], wrTa, relt[:, c0:c0 + cn],
                                 start=True, stop=True)
                r2l.append((c0, cn, r2ps))
            for c0, cn, r2ps in r2l:
                an = min(cn, V * V1 - c0)
                nc.vector.tensor_tensor(
                    out=advu[:, c0:c0 + an], in0=r2ps[:, 0:an],
                    in1=strepA[:, c0:c0 + an], op=mybir.AluOpType.add)
            # bias slot v=25: bg*(rowsumS + 25*a*br + a*Wr@relrowsum)
            c0, cn, r2ps = r2l[-1]
            asum = r2ps[:, V * V1 - c0:V * V1 - c0 + V]
            t1 = smallp.tile([128, V], F32, tag="t1")
            nc.vector.tensor_tensor(out=t1[:], in0=asum, in1=constS2,
                                    op=mybir.AluOpType.add)
            ad3 = advu[:].rearrange("p (u v) -> p u v", v=V1)
            nc.gpsimd.tensor_scalar(
                out=ad3[:, :, V:V1],
                in0=t1[:].rearrange("p (u o) -> p u o", o=1),
                scalar1=bgp, scalar2=None, op0=mybir.AluOpType.mult)

            # ---- adtt [v26, (c, u)] via PE transposes; the copies
            # reshuffle from the per-u PSUM layout to channel-major so
            # the scatter below is 4 plain contiguous 2D DMAs ----
            adtt = adp.tile([128, 128 * V], BF16, tag="adtt")
            a3 = adtt[0:V1, :].rearrange("p (c u) -> p c u", u=V)
            for u0, un in _chunks(V, 8):
                atps = psadt.tile([32, 1024], BF16, tag="atps")
                for ui in range(un):
                    nc.tensor.transpose(
                        atps[0:V1, ui * 128:ui * 128 + 128],
                        advu[:, (u0 + ui) * V1:(u0 + ui + 1) * V1], tident)
                _copy(a3[:, :, u0:u0 + un],
                      atps[0:V1, 0:un * 128].rearrange(
                          "p (u c) -> p c u", c=128))

            # ---- scatter: adtt -> BD block-diag (partition shift) ----
            bd = bds[p % 2]
            for cb in range(4):
                nc.sync.dma_start(
                    bd[cb * 32:cb * 32 + V1, cb * 800:cb * 800 + 800],
                    adtt[0:V1, cb * 800:cb * 800 + 800])

        def stage_out(p):
            # ---- step7: 32 block-diag matmuls, 4 channels each ----
            gt4 = st.pop(("gt4", p))
            bd = bds[p % 2]
            bd_v = bd[:].rearrange("p (c b u) -> p b c u", c=4, b=32, u=V)
            out_sb = outp.tile([128, 2 * CU], BF16, tag="outsb")
            for grp in range(8):
                p7 = ps7.tile([128, 400], F32, tag="p7")
                for j in range(4):
                    b = 4 * grp + j
                    nc.tensor.matmul(
                        p7[:, j * 100:(j + 1) * 100],
                        gt4[:, b * 128:(b + 1) * 128],
                        bd_v[:, b, :, :],
                        start=True, stop=True)
                _copy(out_sb[:, grp * 400:(grp + 1) * 400], p7[:, 0:400])
            # ---- store: per sample, contiguous [t, (b,cb,u)] rows ----
            for s in range(2):
                nc.gpsimd.dma_start(ys_d[2 * p + s],
                                    out_sb[:, s * CU:(s + 1) * CU])

        st[("xp", 0)] = load_x(0)
        st[("xp", 1)] = load_x(1)
        for i in range(NPAIR + 3):
            if i < NPAIR:
                stage_g(i)
            if 0 <= i - 3 < NPAIR:
                stage_out(i - 3)
            if 0 <= i - 1 < NPAIR:
                stage_thph(i - 1)
            if 0 <= i - 2 < NPAIR:
                stage_rel(i - 2)
            if 0 <= i - 2 < NPAIR:
                stage_adv(i - 2)


def _chunks(total, step):
    out = []
    s = 0
    while s < total:
        out.append((s, min(step, total - s)))
        s += step
    return out


def _host_params(A, PA, alpha, Wg, bg, Wth, bth, Wph, bph, Wr, br):
    f = np.float32
    al = np.float32(alpha[0])
    # channel permutation: advu partition p holds channel inv(p)
    inv = (np.arange(128) % 32) * 4 + np.arange(128) // 32
    wgT = np.zeros((128, 128), f)
    wgT[:64, :64] = Wg.T
    wgT[64:, 64:] = Wg.T
    # x_mean: fold 1/T into Wth/Wph lhsT
    wthT = np.zeros((128, 32), f)
    wthT[:64, :16] = Wth.T / T
    wthT[64:, 16:] = Wth.T / T
    wphT = np.zeros((128, 32), f)
    wphT[:64, :16] = Wph.T / T
    wphT[64:, 16:] = Wph.T / T
    wrTa = np.zeros((32, 128), f)
    wrTa[:16, :64] = al * Wr.T
    wrTa[16:, 64:] = al * Wr.T
    wrTa = wrTa[:, inv]                     # permute output channels
    bgg = np.concatenate([bg, bg]).astype(f)
    bgp = bgg[inv % 64].reshape(128, 1)
    bthp = np.concatenate([bth, bth]).astype(f).reshape(32, 1)
    bphp = np.concatenate([bph, bph]).astype(f).reshape(32, 1)
    abr = (al * np.concatenate([br, br])).astype(f)
    abrp = abr[inv % 64].reshape(128, 1)
    S = (A + PA).astype(f)
    # strepA[p, u*26+v] = S[u,v] + alpha*br[inv(p)] for v<25; v=25 -> 0
    sU = np.zeros((V, V1), f)
    sU[:, :V] = S
    strepA = np.tile(sU.reshape(1, -1), (128, 1)).astype(f)
    mask = (np.arange(V * V1) % V1 < V).astype(f).reshape(1, -1)
    strepA = strepA + abrp @ mask
    # constS2[p, u] = rowsumS[u] + 25*alpha*br[inv(p)]
    constS2 = (np.tile(S.sum(axis=1).reshape(1, V), (128, 1))
               + V * abrp).astype(f)
    cA = np.zeros((128, 680), f)
    cA[:, 0:1] = bgp
    cA[0:32, 1:2] = bthp
    cA[0:32, 2:3] = bphp
    cA[:, 4:654] = strepA
    cA[:, 654:679] = constS2
    bf16 = __import__("ml_dtypes").bfloat16
    cB = np.zeros((128, 448), f)
    cB[:, 0:128] = wgT
    cB[:, 128:160] = wthT
    cB[:, 160:192] = wphT
    cB[0:32, 192:320] = wrTa
    cB[:, 320:448] = np.eye(128, dtype=f)
    return dict(constsA=cA, constsB=cB.astype(bf16))


def kernel(**inputs):
    x = np.asarray(inputs["x"], np.float32)
    params = _host_params(
        np.asarray(inputs["A"], np.float32), np.asarray(inputs["PA"], np.float32),
        np.asarray(inputs["alpha"], np.float32), np.asarray(inputs["Wg"], np.float32),
        np.asarray(inputs["bg"], np.float32), np.asarray(inputs["Wth"], np.float32),
        np.asarray(inputs["bth"], np.float32), np.asarray(inputs["Wph"], np.float32),
        np.asarray(inputs["bph"], np.float32), np.asarray(inputs["Wr"], np.float32),
        np.asarray(inputs["br"], np.float32))

    if "nc" not in _cache:
        _cache["nc"] = _build_nc()
    nc = _cache["nc"]

    # upload x v-major, host-converted to bf16 (the g matmul consumes
    # bf16 either way; this halves x HBM traffic and avoids SWDGE casts)
    bf16 = __import__("ml_dtypes").bfloat16
    xv = np.ascontiguousarray(x.transpose(0, 1, 3, 2)).reshape(
        N, C_IN, TV).astype(bf16)
    in_maps = []
    for i in range(NCORES):
        m = {"xs": xv[i * NSH:(i + 1) * NSH]}
        m.update(params)
        in_maps.append(m)

    res = run_bass_kernel_spmd(nc, in_maps, list(range(NCORES)),
                               **_cache.get("run_kwargs", {}))
    # device emits [NSH, T, (b16,cb4,u25)] bf16 per sample (c = 4b+cb)
    out = np.concatenate([np.asarray(res.results[i]["ys"]) for i in range(NCORES)],
                         axis=0)
    out = out.reshape(N, T, 16, 4, V).transpose(0, 2, 3, 1, 4).reshape(
        N, C_OUT, T, V)
    _cache["last_results"] = res
    return np.ascontiguousarray(out, dtype=np.float32)


if __name__ == "__main__":
    nc = _build_nc()
    print("build ok")
